# revision 11
# baseline (speedup 1.0000x reference)
"""AttentionRNN Trainium2 kernel: 8-core SPMD, vocab-split fc projection.

Self-contained: kernel(**inputs) takes full inputs, returns full [B,S,V] output.
Strategy: every core runs the identical embed+xproj+RNN+attention program
(replicated; the RNN scan is latency-bound so data-parallelism would not help),
and computes a 1/8 vocab slice of the final fc projection (the dominant cost,
537 GFLOP total). No collectives needed; host concatenates the vocab slices.
All matmuls in bf16 with f32 PSUM accumulation (measured end-to-end rel err
~3.5e-3 vs f32 reference).
"""
import sys
if '/opt/trn_rl_repo' not in sys.path:
    sys.path.insert(0, '/opt/trn_rl_repo')

import numpy as np
import ml_dtypes

import concourse.bass as bass
import concourse.mybir as mybir
import concourse.tile as tile
from concourse import bacc
from concourse.bass_utils import run_bass_kernel_spmd
from concourse.masks import make_identity

DT = mybir.dt
BF = DT.bfloat16
F32 = DT.float32
BF_NP = ml_dtypes.bfloat16

VOCAB, H, B, S = 32000, 512, 16, 512
NCORES = 8
VS = VOCAB // NCORES          # 4000 vocab rows per core
TOK = B * S                   # 8192 tokens, order tok = t*16 + b
KH = H // 128                 # 4 h-chunks
KD = (2 * H) // 128           # 8 d-chunks of combined
NVB = 8                       # fc vocab chunks per core (7*512 + 416)

# debug dump selector: subset of {"uT", "hsT", "ctxT"}
DEBUG_DUMPS = ()
PHASES = 4


def _vb_width(vb):
    return min(512, VS - vb * 512)


def build_nc(phases=PHASES, dumps=DEBUG_DUMPS):
    nc = bacc.Bacc("TRN2", target_bir_lowering=False, debug=False,
                   num_devices=NCORES)

    emb_bf = nc.dram_tensor("emb_bf", [VOCAB, H], BF, kind="ExternalInput").ap()
    idxw = nc.dram_tensor("idxw", [128, TOK // 16], DT.int16, kind="ExternalInput").ap()
    wxhT = nc.dram_tensor("wxhT", [128, KH * H], BF, kind="ExternalInput").ap()
    whhT = nc.dram_tensor("whhT", [128, KH * H], BF, kind="ExternalInput").ap()
    biasT = nc.dram_tensor("biasT", [128, KH], F32, kind="ExternalInput").ap()
    maskT = nc.dram_tensor("maskT", [128, KH * S], F32, kind="ExternalInput").ap()
    fcwT = nc.dram_tensor("fcwT", [128, KD * VS], BF, kind="ExternalInput").ap()
    fcb = nc.dram_tensor("fcb", [128, VS], F32, kind="ExternalInput").ap()
    if phases >= 4:
        y = nc.dram_tensor("y", [B, S, VS], F32, kind="ExternalOutput").ap()
    dump_aps = {}
    for name in dumps:
        dump_aps[name] = nc.dram_tensor(
            name + "_dump", [128, KH * TOK], BF, kind="ExternalOutput").ap()

    with tile.TileContext(nc) as tc:
        with tc.tile_pool(name="perm", bufs=1) as perm:
            # persistent tiles
            hsT = perm.tile([128, KH * TOK], BF, tag="hsT")
            ident = perm.tile([128, 128], BF, tag="ident")
            make_identity(nc, ident[:])

            # [128, KH, TOK] views; free index = t*16+b
            hsT3 = hsT[:].rearrange("p (k n) -> p k n", k=KH)
            hsT4 = hsT[:].rearrange("p (k t b) -> p k t b", k=KH, b=B)

            # ---------------- phase 1: gather + xproj ----------------
            with tc.tile_pool(name="ph12", bufs=1) as p12:
                xeT = p12.tile([128, KH * TOK], BF, tag="xeT")
                uT = p12.tile([128, KH * TOK], BF, tag="uT")
                wxh_sb = p12.tile([128, KH * H], BF, tag="wxh")
                whh_sb = p12.tile([128, KH * H], BF, tag="whh")
                bias_sb = p12.tile([128, KH], F32, tag="bias")
                idx_sb = p12.tile([128, TOK // 16], DT.int16, tag="idx")
                nc.sync.dma_start(out=wxh_sb[:], in_=wxhT[:])
                nc.sync.dma_start(out=whh_sb[:], in_=whhT[:])
                nc.sync.dma_start(out=bias_sb[:], in_=biasT[:])
                nc.sync.dma_start(out=idx_sb[:], in_=idxw[:])
                NT = 512  # tok chunk for gather + xproj
                NCH = TOK // NT  # 16 chunks
                # chunk-major gather layout: [p, chunk, k, i] = emb[tok, k*128+p]
                xeT4 = xeT[:].rearrange("p (c k n) -> p c k n", c=NCH, k=KH)
                uT3 = uT[:].rearrange("p (k n) -> p k n", k=KH)

                for c in range(NCH):
                    nc.gpsimd.dma_gather(
                        out_ap=xeT4[:, c],
                        in_ap=emb_bf[:],
                        idxs_ap=idx_sb[:, c * (NT // 16):(c + 1) * (NT // 16)],
                        num_idxs=NT,
                        num_idxs_reg=NT,
                        elem_size=H,
                        transpose=True,
                        single_packet=False,
                    )

                with tc.tile_pool(name="ps_x", bufs=4, space="PSUM") as ps_x:
                    for tci in range(NCH):
                        for mg in range(KH):
                            px = ps_x.tile([128, NT], F32, tag="px")
                            for k in range(KH):
                                nc.tensor.matmul(
                                    px[:],
                                    lhsT=wxh_sb[:, k * H + mg * 128:k * H + mg * 128 + 128],
                                    rhs=xeT4[:, tci, k, :],
                                    start=(k == 0), stop=(k == KH - 1),
                                )
                            nc.scalar.activation(
                                uT3[:, mg, tci * NT:(tci + 1) * NT], px[:],
                                mybir.ActivationFunctionType.Identity,
                                bias=bias_sb[:, mg:mg + 1],
                            )
                if "uT" in dump_aps:
                    nc.sync.dma_start(out=dump_aps["uT"][:], in_=uT[:])
                if "xeT" in dump_aps:
                    nc.sync.dma_start(out=dump_aps["xeT"][:], in_=xeT[:])

                # ---------------- phase 2: RNN scan ----------------
                if phases >= 2:
                    with tc.tile_pool(name="ps_r", bufs=2, space="PSUM") as ps_r:
                        for mg in range(KH):
                            nc.scalar.activation(
                                hsT3[:, mg, 0:B], uT3[:, mg, 0:B],
                                mybir.ActivationFunctionType.Tanh)
                        for t in range(1, S):
                            prev = slice((t - 1) * B, t * B)
                            cur = slice(t * B, (t + 1) * B)
                            for mg in range(KH):
                                pm = ps_r.tile([128, B], F32, tag=f"pr{mg}")
                                nc.tensor.matmul(
                                    pm[:], lhsT=ident[:],
                                    rhs=uT3[:, mg, cur],
                                    start=True, stop=False)
                                for k in range(KH):
                                    nc.tensor.matmul(
                                        pm[:],
                                        lhsT=whh_sb[:, k * H + mg * 128:k * H + mg * 128 + 128],
                                        rhs=hsT3[:, k, prev],
                                        start=False, stop=(k == KH - 1))
                                nc.scalar.activation(
                                    hsT3[:, mg, cur], pm[:],
                                    mybir.ActivationFunctionType.Tanh)
            if "hsT" in dump_aps:
                nc.sync.dma_start(out=dump_aps["hsT"][:], in_=hsT[:])

            # ---------------- phase 3: causal attention ----------------
            if phases >= 3:
                with tc.tile_pool(name="ph3", bufs=1) as p3:
                    ctxT = p3.tile([128, KH * TOK], BF, tag="ctxT")
                    mask_sb = p3.tile([128, KH * S], F32, tag="mask")
                    nc.sync.dma_start(out=mask_sb[:], in_=maskT[:])
                    ctxT4 = ctxT[:].rearrange("p (k t b) -> p k t b", k=KH, b=B)
                    with tc.tile_pool(name="p3w", bufs=2) as p3w, \
                         tc.tile_pool(name="ps_s", bufs=2, space="PSUM") as ps_s, \
                         tc.tile_pool(name="ps_t", bufs=4, space="PSUM") as ps_t, \
                         tc.tile_pool(name="ps_c", bufs=2, space="PSUM") as ps_c:
                        for b in range(B):
                            # transpose hsT_b -> hs_b [tk-part, (kh) h]
                            hs_b = p3w.tile([128, KH * H], BF, tag="hs_b")
                            hs_b3 = hs_b[:].rearrange("p (c h) -> p c h", c=KH)
                            for ktk in range(KH):
                                for kh in range(KH):
                                    pt = ps_t.tile([128, 128], BF, tag="pt")
                                    nc.tensor.transpose(
                                        pt[:], hsT4[:, kh, ktk * 128:(ktk + 1) * 128, b],
                                        ident[:])
                                    nc.vector.tensor_copy(
                                        hs_b3[:, ktk, kh * 128:(kh + 1) * 128], pt[:])
                            # scores + softmax per tq tile
                            w_sb = p3w.tile([128, KH * S], BF, tag="w_sb")
                            w_sb3 = w_sb[:].rearrange("p (m n) -> p m n", m=KH)
                            for mq in range(KH):
                                ps = ps_s.tile([128, S], F32, tag="ps")
                                for kh in range(KH):
                                    nc.tensor.matmul(
                                        ps[:],
                                        lhsT=hsT4[:, kh, mq * 128:(mq + 1) * 128, b],
                                        rhs=hsT4[:, kh, :, b],
                                        start=(kh == 0), stop=(kh == KH - 1))
                                ss = p3w.tile([128, S], F32, tag="ss")
                                nc.vector.tensor_tensor(
                                    out=ss[:], in0=ps[:],
                                    in1=mask_sb[:, mq * S:(mq + 1) * S],
                                    op=mybir.AluOpType.add)
                                st = p3w.tile([128, 4], F32, tag="st")
                                nmx, zs, zi = st[:, 0:1], st[:, 1:2], st[:, 2:3]
                                nc.vector.reduce_max(nmx, ss[:],
                                                     axis=mybir.AxisListType.X,
                                                     negate=True)
                                es = p3w.tile([128, S], F32, tag="es")
                                nc.scalar.activation(
                                    es[:], ss[:],
                                    mybir.ActivationFunctionType.Exp,
                                    bias=nmx, accum_out=zs)
                                nc.vector.reciprocal(zi, zs)
                                nc.vector.tensor_scalar_mul(
                                    w_sb3[:, mq, :], es[:], zi)
                            # transpose w -> wT [tk-part, (ktk) tq]
                            wT_sb = p3w.tile([128, KH * S], BF, tag="wT_sb")
                            wT_sb3 = wT_sb[:].rearrange("p (c n) -> p c n", c=KH)
                            for mq in range(KH):
                                for ktk in range(KH):
                                    pt = ps_t.tile([128, 128], BF, tag="pt")
                                    nc.tensor.transpose(
                                        pt[:], w_sb3[:, mq, ktk * 128:(ktk + 1) * 128],
                                        ident[:])
                                    nc.vector.tensor_copy(
                                        wT_sb3[:, ktk, mq * 128:(mq + 1) * 128], pt[:])
                            # contextT chunk = hs_b^T @ wT  -> [h-part, tq]
                            for mh in range(KH):
                                pc = ps_c.tile([128, S], F32, tag="pc")
                                for ktk in range(KH):
                                    nc.tensor.matmul(
                                        pc[:],
                                        lhsT=hs_b3[:, ktk, mh * 128:(mh + 1) * 128],
                                        rhs=wT_sb3[:, ktk, :],
                                        start=(ktk == 0), stop=(ktk == KH - 1))
                                nc.vector.tensor_copy(ctxT4[:, mh, :, b], pc[:])
                    if "ctxT" in dump_aps:
                        nc.sync.dma_start(out=dump_aps["ctxT"][:], in_=ctxT[:])

                    # ---------------- phase 4: fc projection ----------------
                    if phases >= 4:
                        ctxT3 = ctxT[:].rearrange("p (k n) -> p k n", k=KH)
                        y_r = y.rearrange("b (mt dt) v -> mt dt b v", dt=8)
                        with tc.tile_pool(name="fcw", bufs=2) as pfcw, \
                             tc.tile_pool(name="fcb", bufs=1) as pfcb, \
                             tc.tile_pool(name="fco", bufs=4) as pfco, \
                             tc.tile_pool(name="ps_o", bufs=4, space="PSUM") as ps_o:
                            fcb_sb = pfcb.tile([128, VS], F32, tag="fcb")
                            nc.sync.dma_start(out=fcb_sb[:], in_=fcb[:])
                            fcwT3 = fcwT.rearrange("p (k v) -> p k v", k=KD)
                            for vb in range(NVB):
                                vw = _vb_width(vb)
                                fw = pfcw.tile([128, KD * 512], BF, tag="fw")
                                fw3 = fw[:].rearrange("p (k v) -> p k v", k=KD)
                                nc.sync.dma_start(
                                    out=fw3[:, :, 0:vw],
                                    in_=fcwT3[:, :, vb * 512:vb * 512 + vw])
                                for mt in range(TOK // 128):
                                    po = ps_o.tile([128, 512], F32, tag="po")
                                    for k in range(KD):
                                        lhsT = (hsT3[:, k, mt * 128:(mt + 1) * 128]
                                                if k < KH else
                                                ctxT3[:, k - KH, mt * 128:(mt + 1) * 128])
                                        nc.tensor.matmul(
                                            po[:, 0:vw], lhsT=lhsT,
                                            rhs=fw3[:, k, 0:vw],
                                            start=(k == 0), stop=(k == KD - 1))
                                    ob = pfco.tile([128, 512], F32, tag="ob")
                                    nc.vector.tensor_tensor(
                                        out=ob[:, 0:vw], in0=po[:, 0:vw],
                                        in1=fcb_sb[:, vb * 512:vb * 512 + vw],
                                        op=mybir.AluOpType.add)
                                    nc.sync.dma_start(
                                        out=y_r[mt, :, :, vb * 512:vb * 512 + vw],
                                        in_=ob[:, 0:vw])
    nc.compile()
    return nc


# ---------------------------------------------------------------------------
# host side
# ---------------------------------------------------------------------------

def prep_inputs(x, emb, Wxh_w, Wxh_b, Whh_w, Whh_b, fc_w, fc_b):
    """Build per-core in_maps with device layouts."""
    x = np.asarray(x)
    emb = np.asarray(emb, dtype=np.float32)
    Wxh_w = np.asarray(Wxh_w, dtype=np.float32)
    Wxh_b = np.asarray(Wxh_b, dtype=np.float32)
    Whh_w = np.asarray(Whh_w, dtype=np.float32)
    Whh_b = np.asarray(Whh_b, dtype=np.float32)
    fc_w = np.asarray(fc_w, dtype=np.float32)
    fc_b = np.asarray(fc_b, dtype=np.float32)

    emb_bf = np.ascontiguousarray(emb.astype(BF_NP))
    # idx wrapped: flat tok order = t*16+b ; slot j -> [j%16, j//16]
    idx_flat = np.ascontiguousarray(x.T).reshape(-1).astype(np.int64)  # [S*B] t-major
    wrapped = idx_flat.reshape(TOK // 16, 16).T.astype(np.int16)  # [16, TOK//16]
    # replicated across the 8 gpsimd Q7 cores: each reads its own 16-partition group
    idxw = np.ascontiguousarray(np.tile(wrapped, (8, 1)))

    def pack_T(w):  # w [G, H] -> lhsT layout [128, KH*G] : [p, k*G+g] = w[g, k*128+p]
        wT = np.ascontiguousarray(w.T)            # [H, G]
        kh = wT.shape[0] // 128
        return np.ascontiguousarray(
            wT.reshape(kh, 128, wT.shape[1]).transpose(1, 0, 2).reshape(128, -1)
        ).astype(BF_NP)

    wxhT = pack_T(Wxh_w)                          # [128, KH*H]
    whhT = pack_T(Whh_w)
    bias = (Wxh_b + Whh_b).astype(np.float32)
    biasT = np.ascontiguousarray(bias.reshape(KH, 128).T)  # [128, KH]

    p = np.arange(128)[:, None]
    tk = np.arange(S)[None, :]
    maskT = np.zeros((128, KH, S), np.float32)
    for mq in range(KH):
        maskT[:, mq, :] = np.where(tk <= mq * 128 + p, 0.0, -1e30)
    maskT = np.ascontiguousarray(maskT.reshape(128, KH * S))

    base = {
        "emb_bf": emb_bf, "idxw": idxw, "wxhT": wxhT, "whhT": whhT,
        "biasT": biasT, "maskT": maskT,
    }
    in_maps = []
    for c in range(NCORES):
        sl = slice(c * VS, (c + 1) * VS)
        fcwT = pack_T(fc_w[sl])                  # [128, KD*VS]
        fcb_bc = np.ascontiguousarray(
            np.broadcast_to(fc_b[sl].astype(np.float32), (128, VS)))
        m = dict(base)
        m["fcwT"] = fcwT
        m["fcb"] = fcb_bc
        in_maps.append(m)
    return in_maps


_NC_CACHE = {}


def get_nc(phases=PHASES, dumps=DEBUG_DUMPS):
    key = (phases, tuple(dumps))
    if key not in _NC_CACHE:
        _NC_CACHE[key] = build_nc(phases, dumps)
    return _NC_CACHE[key]


def kernel(x, emb, Wxh_w, Wxh_b, Whh_w, Whh_b, fc_w, fc_b):
    nc = get_nc()
    in_maps = prep_inputs(x, emb, Wxh_w, Wxh_b, Whh_w, Whh_b, fc_w, fc_b)
    res = run_bass_kernel_spmd(nc, in_maps, list(range(NCORES)))
    y = np.concatenate([res.results[c]["y"] for c in range(NCORES)], axis=2)
    return np.ascontiguousarray(y.astype(np.float32))


# revision 12
# speedup vs baseline: 45.6735x; 45.6735x over previous
"""AttentionRNN Trainium2 kernel: 8-core SPMD, vocab-split fc projection.

Self-contained: kernel(**inputs) takes full inputs, returns full [B,S,V] output.
Strategy: every core runs the identical embed+xproj+RNN+attention program
(replicated; the RNN scan is latency-bound so data-parallelism would not help),
and computes a 1/8 vocab slice of the final fc projection (the dominant cost,
537 GFLOP total). No collectives needed; host concatenates the vocab slices.
All matmuls in bf16 with f32 PSUM accumulation (measured end-to-end rel err
~3.5e-3 vs f32 reference).
"""
import sys
if '/opt/trn_rl_repo' not in sys.path:
    sys.path.insert(0, '/opt/trn_rl_repo')

import numpy as np
import ml_dtypes

import concourse.bass as bass
import concourse.mybir as mybir
import concourse.tile as tile
from concourse import bacc
from concourse.bass_utils import run_bass_kernel_spmd
from concourse.masks import make_identity

DT = mybir.dt
BF = DT.bfloat16
F32 = DT.float32
BF_NP = ml_dtypes.bfloat16

VOCAB, H, B, S = 32000, 512, 16, 512
NCORES = 8
VS = VOCAB // NCORES          # 4000 vocab rows per core
TOK = B * S                   # 8192 tokens, order tok = t*16 + b
KH = H // 128                 # 4 h-chunks
KD = (2 * H) // 128           # 8 d-chunks of combined
NVB = 8                       # fc vocab chunks per core (7*512 + 416)

# debug dump selector: subset of {"uT", "hsT", "ctxT"}
DEBUG_DUMPS = ()
PHASES = 4


def _vb_width(vb):
    return min(512, VS - vb * 512)


def build_nc(phases=PHASES, dumps=DEBUG_DUMPS, repeat=1):
    nc = bacc.Bacc("TRN2", target_bir_lowering=False, debug=False,
                   num_devices=NCORES)

    emb_bf = nc.dram_tensor("emb_bf", [VOCAB, H], BF, kind="ExternalInput").ap()
    idxw = nc.dram_tensor("idxw", [128, TOK // 16], DT.int16, kind="ExternalInput").ap()
    wxhT = nc.dram_tensor("wxhT", [128, KH * H], BF, kind="ExternalInput").ap()
    whhT = nc.dram_tensor("whhT", [128, KH * H], BF, kind="ExternalInput").ap()
    biasT = nc.dram_tensor("biasT", [128, KH], F32, kind="ExternalInput").ap()
    maskT = nc.dram_tensor("maskT", [128, KH * S], F32, kind="ExternalInput").ap()
    fcwT = nc.dram_tensor("fcwT", [128, KD * VS], BF, kind="ExternalInput").ap()
    fcb = nc.dram_tensor("fcb", [128, VS], F32, kind="ExternalInput").ap()
    if phases >= 4:
        y = nc.dram_tensor("y", [B, S, VS], F32, kind="ExternalOutput").ap()
    dump_aps = {}
    for name in dumps:
        dump_aps[name] = nc.dram_tensor(
            name + "_dump", [128, KH * TOK], BF, kind="ExternalOutput").ap()

    with tile.TileContext(nc) as tc:
      for _rep in range(repeat):
        with tc.tile_pool(name="perm", bufs=1) as perm:
            # persistent tiles
            hsT = perm.tile([128, KH * TOK], BF, tag="hsT")
            ident = perm.tile([128, 128], BF, tag="ident")
            make_identity(nc, ident[:])

            # [128, KH, TOK] views; free index = t*16+b
            hsT3 = hsT[:].rearrange("p (k n) -> p k n", k=KH)
            hsT4 = hsT[:].rearrange("p (k t b) -> p k t b", k=KH, b=B)

            # ---------------- phase 1: gather + xproj ----------------
            with tc.tile_pool(name="ph12", bufs=1) as p12:
                xeT = p12.tile([128, KH * TOK], BF, tag="xeT")
                uT = p12.tile([128, KH * TOK], BF, tag="uT")
                wxh_sb = p12.tile([128, KH * H], BF, tag="wxh")
                whh_sb = p12.tile([128, KH * H], BF, tag="whh")
                bias_sb = p12.tile([128, KH], F32, tag="bias")
                idx_sb = p12.tile([128, TOK // 16], DT.int16, tag="idx")
                nc.sync.dma_start(out=wxh_sb[:], in_=wxhT[:])
                nc.sync.dma_start(out=whh_sb[:], in_=whhT[:])
                nc.sync.dma_start(out=bias_sb[:], in_=biasT[:])
                nc.sync.dma_start(out=idx_sb[:], in_=idxw[:])
                NT = 512  # tok chunk for gather + xproj
                NCH = TOK // NT  # 16 chunks
                # chunk-major gather layout: [p, chunk, k, i] = emb[tok, k*128+p]
                xeT4 = xeT[:].rearrange("p (c k n) -> p c k n", c=NCH, k=KH)
                uT3 = uT[:].rearrange("p (k n) -> p k n", k=KH)

                for c in range(NCH):
                    nc.gpsimd.dma_gather(
                        out_ap=xeT4[:, c],
                        in_ap=emb_bf[:],
                        idxs_ap=idx_sb[:, c * (NT // 16):(c + 1) * (NT // 16)],
                        num_idxs=NT,
                        num_idxs_reg=NT,
                        elem_size=H,
                        transpose=True,
                        single_packet=False,
                    )

                with tc.tile_pool(name="ps_x", bufs=4, space="PSUM") as ps_x:
                    for tci in range(NCH):
                        for mg in range(KH):
                            px = ps_x.tile([128, NT], F32, tag="px")
                            for k in range(KH):
                                nc.tensor.matmul(
                                    px[:],
                                    lhsT=wxh_sb[:, k * H + mg * 128:k * H + mg * 128 + 128],
                                    rhs=xeT4[:, tci, k, :],
                                    start=(k == 0), stop=(k == KH - 1),
                                )
                            nc.scalar.activation(
                                uT3[:, mg, tci * NT:(tci + 1) * NT], px[:],
                                mybir.ActivationFunctionType.Identity,
                                bias=bias_sb[:, mg:mg + 1],
                            )
                if "uT" in dump_aps:
                    nc.sync.dma_start(out=dump_aps["uT"][:], in_=uT[:])
                if "xeT" in dump_aps:
                    nc.sync.dma_start(out=dump_aps["xeT"][:], in_=xeT[:])

                # ---------------- phase 2: RNN scan ----------------
                if phases >= 2:
                    with tc.tile_pool(name="ps_r", bufs=2, space="PSUM") as ps_r:
                        for mg in range(KH):
                            nc.scalar.activation(
                                hsT3[:, mg, 0:B], uT3[:, mg, 0:B],
                                mybir.ActivationFunctionType.Tanh)
                        for t in range(1, S):
                            prev = slice((t - 1) * B, t * B)
                            cur = slice(t * B, (t + 1) * B)
                            for mg in range(KH):
                                pm = ps_r.tile([128, B], F32, tag=f"pr{mg}")
                                nc.tensor.matmul(
                                    pm[:], lhsT=ident[:],
                                    rhs=uT3[:, mg, cur],
                                    start=True, stop=False)
                                for k in range(KH):
                                    nc.tensor.matmul(
                                        pm[:],
                                        lhsT=whh_sb[:, k * H + mg * 128:k * H + mg * 128 + 128],
                                        rhs=hsT3[:, k, prev],
                                        start=False, stop=(k == KH - 1))
                                nc.scalar.activation(
                                    hsT3[:, mg, cur], pm[:],
                                    mybir.ActivationFunctionType.Tanh)
            if "hsT" in dump_aps:
                nc.sync.dma_start(out=dump_aps["hsT"][:], in_=hsT[:])

            # ---------------- phase 3: causal attention ----------------
            if phases >= 3:
                with tc.tile_pool(name="ph3", bufs=1) as p3:
                    ctxT = p3.tile([128, KH * TOK], BF, tag="ctxT")
                    mask_sb = p3.tile([128, KH * S], F32, tag="mask")
                    nc.sync.dma_start(out=mask_sb[:], in_=maskT[:])
                    ctxT4 = ctxT[:].rearrange("p (k t b) -> p k t b", k=KH, b=B)
                    with tc.tile_pool(name="p3w", bufs=2) as p3w, \
                         tc.tile_pool(name="ps_s", bufs=2, space="PSUM") as ps_s, \
                         tc.tile_pool(name="ps_t", bufs=4, space="PSUM") as ps_t, \
                         tc.tile_pool(name="ps_c", bufs=2, space="PSUM") as ps_c:
                        for b in range(B):
                            # transpose hsT_b -> hs_b [tk-part, (kh) h]
                            hs_b = p3w.tile([128, KH * H], BF, tag="hs_b")
                            hs_b3 = hs_b[:].rearrange("p (c h) -> p c h", c=KH)
                            for ktk in range(KH):
                                for kh in range(KH):
                                    pt = ps_t.tile([128, 128], BF, tag="pt")
                                    nc.tensor.transpose(
                                        pt[:], hsT4[:, kh, ktk * 128:(ktk + 1) * 128, b],
                                        ident[:])
                                    nc.vector.tensor_copy(
                                        hs_b3[:, ktk, kh * 128:(kh + 1) * 128], pt[:])
                            # scores + softmax per tq tile
                            w_sb = p3w.tile([128, KH * S], BF, tag="w_sb")
                            w_sb3 = w_sb[:].rearrange("p (m n) -> p m n", m=KH)
                            for mq in range(KH):
                                ps = ps_s.tile([128, S], F32, tag="ps")
                                for kh in range(KH):
                                    nc.tensor.matmul(
                                        ps[:],
                                        lhsT=hsT4[:, kh, mq * 128:(mq + 1) * 128, b],
                                        rhs=hsT4[:, kh, :, b],
                                        start=(kh == 0), stop=(kh == KH - 1))
                                ss = p3w.tile([128, S], F32, tag="ss")
                                nc.vector.tensor_tensor(
                                    out=ss[:], in0=ps[:],
                                    in1=mask_sb[:, mq * S:(mq + 1) * S],
                                    op=mybir.AluOpType.add)
                                st = p3w.tile([128, 4], F32, tag="st")
                                nmx, zs, zi = st[:, 0:1], st[:, 1:2], st[:, 2:3]
                                nc.vector.reduce_max(nmx, ss[:],
                                                     axis=mybir.AxisListType.X,
                                                     negate=True)
                                es = p3w.tile([128, S], F32, tag="es")
                                nc.scalar.activation(
                                    es[:], ss[:],
                                    mybir.ActivationFunctionType.Exp,
                                    bias=nmx, accum_out=zs)
                                nc.vector.reciprocal(zi, zs)
                                nc.vector.tensor_scalar_mul(
                                    w_sb3[:, mq, :], es[:], zi)
                            # transpose w -> wT [tk-part, (ktk) tq]
                            wT_sb = p3w.tile([128, KH * S], BF, tag="wT_sb")
                            wT_sb3 = wT_sb[:].rearrange("p (c n) -> p c n", c=KH)
                            for mq in range(KH):
                                for ktk in range(KH):
                                    pt = ps_t.tile([128, 128], BF, tag="pt")
                                    nc.tensor.transpose(
                                        pt[:], w_sb3[:, mq, ktk * 128:(ktk + 1) * 128],
                                        ident[:])
                                    nc.vector.tensor_copy(
                                        wT_sb3[:, ktk, mq * 128:(mq + 1) * 128], pt[:])
                            # contextT chunk = hs_b^T @ wT  -> [h-part, tq]
                            for mh in range(KH):
                                pc = ps_c.tile([128, S], F32, tag="pc")
                                for ktk in range(KH):
                                    nc.tensor.matmul(
                                        pc[:],
                                        lhsT=hs_b3[:, ktk, mh * 128:(mh + 1) * 128],
                                        rhs=wT_sb3[:, ktk, :],
                                        start=(ktk == 0), stop=(ktk == KH - 1))
                                nc.vector.tensor_copy(ctxT4[:, mh, :, b], pc[:])
                    if "ctxT" in dump_aps:
                        nc.sync.dma_start(out=dump_aps["ctxT"][:], in_=ctxT[:])

                    # ---------------- phase 4: fc projection ----------------
                    if phases >= 4:
                        ctxT3 = ctxT[:].rearrange("p (k n) -> p k n", k=KH)
                        y_r = y.rearrange("b (mt dt) v -> mt dt b v", dt=8)
                        with tc.tile_pool(name="fcw", bufs=2) as pfcw, \
                             tc.tile_pool(name="fcb", bufs=1) as pfcb, \
                             tc.tile_pool(name="fco", bufs=4) as pfco, \
                             tc.tile_pool(name="ps_o", bufs=4, space="PSUM") as ps_o:
                            fcb_sb = pfcb.tile([128, VS], F32, tag="fcb")
                            nc.sync.dma_start(out=fcb_sb[:], in_=fcb[:])
                            fcwT3 = fcwT.rearrange("p (k v) -> p k v", k=KD)
                            for vb in range(NVB):
                                vw = _vb_width(vb)
                                fw = pfcw.tile([128, KD * 512], BF, tag="fw")
                                fw3 = fw[:].rearrange("p (k v) -> p k v", k=KD)
                                nc.sync.dma_start(
                                    out=fw3[:, :, 0:vw],
                                    in_=fcwT3[:, :, vb * 512:vb * 512 + vw])
                                for mt in range(TOK // 128):
                                    po = ps_o.tile([128, 512], F32, tag="po")
                                    for k in range(KD):
                                        lhsT = (hsT3[:, k, mt * 128:(mt + 1) * 128]
                                                if k < KH else
                                                ctxT3[:, k - KH, mt * 128:(mt + 1) * 128])
                                        nc.tensor.matmul(
                                            po[:, 0:vw], lhsT=lhsT,
                                            rhs=fw3[:, k, 0:vw],
                                            start=(k == 0), stop=(k == KD - 1))
                                    ob = pfco.tile([128, 512], F32, tag="ob")
                                    nc.vector.tensor_tensor(
                                        out=ob[:, 0:vw], in0=po[:, 0:vw],
                                        in1=fcb_sb[:, vb * 512:vb * 512 + vw],
                                        op=mybir.AluOpType.add)
                                    nc.sync.dma_start(
                                        out=y_r[mt, :, :, vb * 512:vb * 512 + vw],
                                        in_=ob[:, 0:vw])
    nc.compile()
    return nc


# ---------------------------------------------------------------------------
# host side
# ---------------------------------------------------------------------------

def prep_inputs(x, emb, Wxh_w, Wxh_b, Whh_w, Whh_b, fc_w, fc_b):
    """Build per-core in_maps with device layouts."""
    x = np.asarray(x)
    emb = np.asarray(emb, dtype=np.float32)
    Wxh_w = np.asarray(Wxh_w, dtype=np.float32)
    Wxh_b = np.asarray(Wxh_b, dtype=np.float32)
    Whh_w = np.asarray(Whh_w, dtype=np.float32)
    Whh_b = np.asarray(Whh_b, dtype=np.float32)
    fc_w = np.asarray(fc_w, dtype=np.float32)
    fc_b = np.asarray(fc_b, dtype=np.float32)

    emb_bf = np.ascontiguousarray(emb.astype(BF_NP))
    # idx wrapped: flat tok order = t*16+b ; slot j -> [j%16, j//16]
    idx_flat = np.ascontiguousarray(x.T).reshape(-1).astype(np.int64)  # [S*B] t-major
    wrapped = idx_flat.reshape(TOK // 16, 16).T.astype(np.int16)  # [16, TOK//16]
    # replicated across the 8 gpsimd Q7 cores: each reads its own 16-partition group
    idxw = np.ascontiguousarray(np.tile(wrapped, (8, 1)))

    def pack_T(w):  # w [G, H] -> lhsT layout [128, KH*G] : [p, k*G+g] = w[g, k*128+p]
        wT = np.ascontiguousarray(w.T)            # [H, G]
        kh = wT.shape[0] // 128
        return np.ascontiguousarray(
            wT.reshape(kh, 128, wT.shape[1]).transpose(1, 0, 2).reshape(128, -1)
        ).astype(BF_NP)

    wxhT = pack_T(Wxh_w)                          # [128, KH*H]
    whhT = pack_T(Whh_w)
    bias = (Wxh_b + Whh_b).astype(np.float32)
    biasT = np.ascontiguousarray(bias.reshape(KH, 128).T)  # [128, KH]

    p = np.arange(128)[:, None]
    tk = np.arange(S)[None, :]
    maskT = np.zeros((128, KH, S), np.float32)
    for mq in range(KH):
        maskT[:, mq, :] = np.where(tk <= mq * 128 + p, 0.0, -1e30)
    maskT = np.ascontiguousarray(maskT.reshape(128, KH * S))

    base = {
        "emb_bf": emb_bf, "idxw": idxw, "wxhT": wxhT, "whhT": whhT,
        "biasT": biasT, "maskT": maskT,
    }
    in_maps = []
    for c in range(NCORES):
        sl = slice(c * VS, (c + 1) * VS)
        fcwT = pack_T(fc_w[sl])                  # [128, KD*VS]
        fcb_bc = np.ascontiguousarray(
            np.broadcast_to(fc_b[sl].astype(np.float32), (128, VS)))
        m = dict(base)
        m["fcwT"] = fcwT
        m["fcb"] = fcb_bc
        in_maps.append(m)
    return in_maps


_NC_CACHE = {}


def get_nc(phases=PHASES, dumps=DEBUG_DUMPS):
    key = (phases, tuple(dumps))
    if key not in _NC_CACHE:
        _NC_CACHE[key] = build_nc(phases, dumps)
    return _NC_CACHE[key]


def kernel(x, emb, Wxh_w, Wxh_b, Whh_w, Whh_b, fc_w, fc_b):
    nc = get_nc()
    in_maps = prep_inputs(x, emb, Wxh_w, Wxh_b, Whh_w, Whh_b, fc_w, fc_b)
    res = run_bass_kernel_spmd(nc, in_maps, list(range(NCORES)))
    y = np.concatenate([res.results[c]["y"] for c in range(NCORES)], axis=2)
    return np.ascontiguousarray(y.astype(np.float32))
